# revision 36
# baseline (speedup 1.0000x reference)
"""Trainium2 Bass kernel for gated multi-head attention (nn_Attention_19490561589255).

Reference computation (B=2, S=2048, E=256, H=8, D=32):
    xn = LayerNorm(x)                       (no affine)
    q,k,v = split_heads(xn @ W{q,k,v}.T);  q *= 1/sqrt(D)
    gate  = sigmoid(split_heads(xn @ Wg.T + bg))
    logits = q @ k.T + attn_bias;  logits[mask<0.5 keys] = -2^15
    out = gate * softmax(logits) @ v;  out = merge_heads(out) @ Wo.T + bo

Sharding: each of the 8 cores owns a 256-row query block (all batches, all
heads).  k/v/gate weights + full x are replicated (tiny); the 268 MB
attn_bias is sliced by query block and passed pre-transposed so logits are
computed as logitsT[j, i] (keys on partitions).  In that layout:
  - the key mask is applied by zeroing masked rows of the [v | ones]
    stationary operand, so masked keys contribute 0 to both the attn@v
    numerator and the softmax denominator (exactly the -inf semantics),
  - the softmax denominator falls out of the attn@v matmul via the extra
    ones-column (no max-subtraction needed: logits ~ N(0,2), f32-exp safe),
  - attn_bias is injected into PSUM by an identity-stationary matmul so the
    PE accumulation hardware performs the big bias add; adjacent key tiles
    are paired into one [128,512] PSUM bank (one inject + one exp per pair),
  - matmul operands are bf16 (PSUM accumulation stays f32),
  - each core writes its own output rows: NO collectives, host gather is a
    pure concatenation.
"""

import os
import sys

sys.path.insert(0, "/opt/trn_rl_repo")

import ml_dtypes
import numpy as np

import concourse.bass as bass
import concourse.mybir as mybir
import concourse.tile as tile
from concourse import bacc
from concourse.bass_utils import run_bass_kernel_spmd

F32 = mybir.dt.float32
BF16 = mybir.dt.bfloat16
AF = mybir.ActivationFunctionType
ALU = mybir.AluOpType

B, S, E = 2, 2048, 256
H, D = 8, 32
HD = H * D          # 256
P = 128
NC = 8
SHARD = S // NC     # 256 query rows per core
ET = E // P         # 2 e-tiles
KT = HD // P        # 2 hd-tiles
JT = S // P         # 16 key tiles
INF = 2.0 ** 15
EPS = 1e-5

_CACHE = {}


def _build():
    nc = bacc.Bacc(
        "TRN2", target_bir_lowering=False, debug=False, enable_asserts=False,
        num_devices=NC,
    )
    x_d = nc.dram_tensor("x", [P, B * JT * E], F32, kind="ExternalInput").ap()
    xq_d = nc.dram_tensor("xq", [P, B * 2 * E], F32, kind="ExternalInput").ap()
    mk_d = nc.dram_tensor("maskT", [P, B * JT], F32, kind="ExternalInput").ap()
    bias_d = nc.dram_tensor("biasT", [B, H, P, JT * SHARD], BF16, kind="ExternalInput").ap()
    wq_d = nc.dram_tensor("wqt", [P, ET * HD], BF16, kind="ExternalInput").ap()
    wk_d = nc.dram_tensor("wkt", [P, ET * HD], BF16, kind="ExternalInput").ap()
    wv_d = nc.dram_tensor("wvt", [P, ET * HD], BF16, kind="ExternalInput").ap()
    wg_d = nc.dram_tensor("wgt", [P, ET * HD], BF16, kind="ExternalInput").ap()
    wo_d = nc.dram_tensor("wot", [P, KT * E], BF16, kind="ExternalInput").ap()
    bg_d = nc.dram_tensor("bg", [HD], F32, kind="ExternalInput").ap()
    id_d = nc.dram_tensor("ident", [P, P], F32, kind="ExternalInput").ap()
    idb_d = nc.dram_tensor("identbf", [P, P], BF16, kind="ExternalInput").ap()
    er_d = nc.dram_tensor("erow", [D, P], F32, kind="ExternalInput").ap()
    erb_d = nc.dram_tensor("erowbf", [D, P], BF16, kind="ExternalInput").ap()
    bo32_d = nc.dram_tensor("bo32bf", [D, E], BF16, kind="ExternalInput").ap()
    out_d = nc.dram_tensor("out", [B, SHARD, E], F32, kind="ExternalOutput").ap()

    from contextlib import ExitStack

    with ExitStack() as ctx:
        tc = ctx.enter_context(tile.TileContext(nc))
        cpool = ctx.enter_context(tc.tile_pool(name="consts", bufs=1))
        wpool = ctx.enter_context(tc.tile_pool(name="wts", bufs=1))
        xpool = ctx.enter_context(tc.tile_pool(name="xload", bufs=4))
        spool = ctx.enter_context(tc.tile_pool(name="stats", bufs=8))
        xnpool = ctx.enter_context(tc.tile_pool(name="xn", bufs=4))
        xnTpool = ctx.enter_context(tc.tile_pool(name="xnT", bufs=2))
        kTpool = ctx.enter_context(tc.tile_pool(name="kT", bufs=2))
        vpool = ctx.enter_context(tc.tile_pool(name="v33", bufs=2))
        qgpool = ctx.enter_context(tc.tile_pool(name="qg", bufs=2))
        bpool = ctx.enter_context(tc.tile_pool(name="bias", bufs=4))
        prpool = ctx.enter_context(tc.tile_pool(name="probs", bufs=5))
        ogpool = ctx.enter_context(tc.tile_pool(name="outg", bufs=2))
        fpool = ctx.enter_context(tc.tile_pool(name="fin", bufs=2))
        khpool = ctx.enter_context(tc.tile_pool(name="khst", bufs=2))
        smpool = ctx.enter_context(tc.tile_pool(name="small", bufs=2))
        plpool = ctx.enter_context(tc.tile_pool(name="pl", bufs=2, space="PSUM"))
        popool = ctx.enter_context(tc.tile_pool(name="po", bufs=4, space="PSUM"))
        if True:
            # ---- constants / weights ----
            ident = cpool.tile([P, P], F32)
            nc.sync.dma_start(out=ident[:], in_=id_d[:])
            identbf = cpool.tile([P, P], BF16, tag="identbf")
            nc.sync.dma_start(out=identbf[:], in_=idb_d[:])
            erow = cpool.tile([2 * D, P], F32, tag="erow")
            nc.sync.dma_start(out=erow[D:2 * D, :], in_=er_d[:])
            erowbf = cpool.tile([D, P], BF16, tag="erowbf")
            nc.sync.dma_start(out=erowbf[:], in_=erb_d[:])
            bo32 = cpool.tile([D, E], BF16, tag="bo32")
            nc.sync.dma_start(out=bo32[:], in_=bo32_d[:])
            bgp = cpool.tile([D, H], F32)
            nc.sync.dma_start(out=bgp[:], in_=bg_d.rearrange("(h p) -> p h", p=D))
            epsc = cpool.tile([P, 1], F32, tag="epsc")
            nc.vector.memset(epsc[:], EPS)
            zeroc = cpool.tile([P, 1], F32, tag="zeroc")
            nc.vector.memset(zeroc[:], 0.0)

            wq = wpool.tile([P, ET, HD], BF16, tag="wq")
            wk = wpool.tile([P, ET, HD], BF16, tag="wk")
            wv = wpool.tile([P, ET, HD], BF16, tag="wv")
            wg = wpool.tile([P, ET, HD], BF16, tag="wg")
            wo = wpool.tile([P, KT, E], BF16, tag="wo")
            for t, d in ((wq, wq_d), (wk, wk_d), (wv, wv_d), (wg, wg_d), (wo, wo_d)):
                nc.sync.dma_start(out=t[:], in_=d.rearrange("p (et m) -> p et m", m=HD))

            # key-mask multiplier: 1.0 where mask>=0.5 else 0.0
            mk = cpool.tile([P, B * JT], F32, tag="mk")
            nc.sync.dma_start(out=mk[:], in_=mk_d[:])
            maskm = cpool.tile([P, B * JT], F32, tag="maskm")
            nc.vector.tensor_scalar(maskm[:], mk[:], 0.5, None, ALU.is_ge)

            def layernorm_rows(src_groups, dst_tile):
                """LN groups of [128, G, E] rows, transpose into dst_tile."""
                for gi, (src_ap, g) in enumerate(src_groups):
                    xt = xpool.tile([P, 4, E], F32, tag="xt")
                    nc.sync.dma_start(out=xt[:, 0:g, :], in_=src_ap)
                    s1 = spool.tile([P, 4], F32, tag="s1")
                    s2 = spool.tile([P, 4], F32, tag="s2")
                    sq = xpool.tile([P, 4, E], F32, tag="sq")
                    nc.vector.tensor_reduce(
                        s1[:, 0:g], xt[:, 0:g, :], axis=mybir.AxisListType.X,
                        op=ALU.add,
                    )
                    for r in range(g):
                        nc.scalar.activation(
                            sq[:, r, :], xt[:, r, :], AF.Square,
                            accum_out=s2[:, r:r + 1],
                        )
                    mu = spool.tile([P, 4], F32, tag="mu")
                    nc.vector.tensor_scalar_mul(mu[:, 0:g], s1[:, 0:g], 1.0 / E)
                    ex2 = spool.tile([P, 4], F32, tag="ex2")
                    nc.vector.tensor_scalar_mul(ex2[:, 0:g], s2[:, 0:g], 1.0 / E)
                    nvar = spool.tile([P, 4], F32, tag="nvar")
                    # nvar = mu*mu - ex2  (negated variance)
                    nc.vector.tensor_tensor(
                        nvar[:, 0:g], mu[:, 0:g], mu[:, 0:g], ALU.mult
                    )
                    nc.vector.tensor_sub(nvar[:, 0:g], nvar[:, 0:g], ex2[:, 0:g])
                    sd = spool.tile([P, 4], F32, tag="sd")
                    # sd = sqrt(-nvar + eps) = sqrt(var + eps)
                    nc.scalar.activation(
                        sd[:, 0:g], nvar[:, 0:g], AF.Sqrt, bias=epsc[:, 0:1],
                        scale=-1.0,
                    )
                    rs = spool.tile([P, 4], F32, tag="rs")
                    nc.vector.reciprocal(rs[:, 0:g], sd[:, 0:g])
                    xnt = xnpool.tile([P, 4, E], BF16, tag="xnt")
                    # xn = (x - mu) * rs   (free-axis broadcast of [P,g] stats)
                    xc = xpool.tile([P, 4, E], F32, tag="xc")
                    nc.vector.tensor_sub(
                        xc[:, 0:g, :], xt[:, 0:g, :],
                        mu[:, 0:g, None].broadcast_to([P, g, E]),
                    )
                    nc.vector.tensor_mul(
                        xnt[:, 0:g, :], xc[:, 0:g, :],
                        rs[:, 0:g, None].broadcast_to([P, g, E]),
                    )
                    for r in range(g):
                        col = (gi * 4 + r) * P
                        for et in range(ET):
                            pt = plpool.tile([P, 1024], BF16, tag="pl", name="pt")
                            nc.tensor.transpose(
                                pt[:, 0:P], xnt[:, r, et * P:(et + 1) * P],
                                identbf[:],
                            )
                            nc.scalar.copy(
                                dst_tile[:, et, col:col + P], pt[:, 0:P]
                            )

            x_rows = x_d.rearrange("p (r e) -> p r e", e=E)
            xq_rows = xq_d.rearrange("p (r e) -> p r e", e=E)

            # ---- phases A+B per batch: LN+transpose then projections ----
            xnT = {}
            xqnT = {}
            kT = {}
            v33 = {}
            qh = {}
            gh = {}
            for b in range(B):
                xnT[b] = xnTpool.tile([P, ET, S], BF16, tag="xnT", name="xnT")
                layernorm_rows(
                    [(x_rows[:, b * JT + 4 * gi:b * JT + 4 * (gi + 1), :], 4)
                     for gi in range(JT // 4)],
                    xnT[b],
                )
                xqnT[b] = xnTpool.tile([P, ET, SHARD], BF16, tag="xqnT", name="xqnT")
                layernorm_rows(
                    [(xq_rows[:, b * 2:b * 2 + 2, :], 2)],
                    xqnT[b],
                )

                kT[b] = kTpool.tile([P, KT, S], BF16, tag="kT", name="kT")
                for kt in range(KT):
                    for nb in range(S // 512):
                        pp = plpool.tile([P, 1024], F32, tag="pl", name="pp")
                        for et in range(ET):
                            nc.tensor.matmul(
                                pp[:, 0:512], wk[:, et, kt * P:(kt + 1) * P],
                                xnT[b][:, et, nb * 512:(nb + 1) * 512],
                                start=(et == 0), stop=(et == ET - 1),
                            )
                        nc.vector.tensor_copy(
                            kT[b][:, kt, nb * 512:(nb + 1) * 512], pp[:, 0:512]
                        )

                v33[b] = vpool.tile([P, JT, H * 33], BF16, tag="v", name="v33")
                for jt in range(JT):
                    mcol = b * JT + jt
                    pp = plpool.tile([P, 1024], F32, tag="pl", name="pp")
                    for et in range(ET):
                        nc.tensor.matmul(
                            pp[:, 0:HD], xnT[b][:, et, jt * P:(jt + 1) * P],
                            wv[:, et, :], start=(et == 0), stop=(et == ET - 1),
                        )
                    v3 = v33[b][:, jt, :].rearrange("p (h x) -> p h x", x=33)
                    # masked v into cols 0..31 (ACT copy scaled by mask multiplier)
                    nc.scalar.activation(
                        v3[:, :, 0:D],
                        pp[:, 0:HD].rearrange("p (h x) -> p h x", x=D),
                        AF.Copy, scale=maskm[:, mcol:mcol + 1],
                    )
                    # trailing ones-column = mask multiplier (1 live / 0 masked)
                    nc.vector.tensor_copy(
                        v3[:, :, D:D + 1],
                        maskm[:, None, mcol:mcol + 1].broadcast_to([P, H, 1]),
                    )

                qh[b] = qgpool.tile([D, H, SHARD], BF16, tag="q", name="qh")
                gh[b] = qgpool.tile([D, H, SHARD], BF16, tag="g", name="gh")
                for h in range(H):
                    pp = plpool.tile([P, 1024], F32, tag="pl", name="pp")
                    for et in range(ET):
                        nc.tensor.matmul(
                            pp[0:D, 0:SHARD], wq[:, et, h * D:(h + 1) * D],
                            xqnT[b][:, et, :], start=(et == 0), stop=(et == ET - 1),
                        )
                    nc.scalar.copy(qh[b][:, h, :], pp[0:D, 0:SHARD])
                    pp2 = plpool.tile([P, 1024], F32, tag="pl", name="pp2")
                    for et in range(ET):
                        nc.tensor.matmul(
                            pp2[0:D, 0:SHARD], wg[:, et, h * D:(h + 1) * D],
                            xqnT[b][:, et, :], start=(et == 0), stop=(et == ET - 1),
                        )
                    nc.scalar.activation(
                        gh[b][:, h, :], pp2[0:D, 0:SHARD], AF.Sigmoid,
                        bias=bgp[:, h:h + 1],
                    )

            # ---- phase C: attention ----
            outg = {}
            for b in range(B):
                outg[b] = ogpool.tile([P, KT, SHARD], BF16, tag="og", name="outg")
                bias_rows = bias_d[b].rearrange("h p (g i) -> h p g i", i=SHARD)
                for h in range(H):
                    kt_i, ro = h // 4, (h % 4) * D
                    kh = khpool.tile([D, S], BF16, tag="kh")
                    nc.vector.tensor_copy(kh[:], kT[b][ro:ro + D, kt_i, :])
                    bt = bpool.tile([P, JT, SHARD], BF16, tag="bias")
                    beng = nc.gpsimd if h % 2 == 0 else nc.scalar
                    beng.dma_start(out=bt[:], in_=bias_rows[h])
                    poA = popool.tile([D + 1, SHARD], F32, tag="po", name="poA")
                    poB = popool.tile([D + 1, SHARD], F32, tag="po", name="poB")
                    for qd in range(JT // 4):
                        jt0 = qd * 4
                        pl = plpool.tile([P, 1024], F32, tag="pl")
                        nc.tensor.matmul(
                            pl[:, 0:512], identbf[:],
                            bt[:, jt0:jt0 + 2, :], start=True, stop=False,
                        )
                        nc.tensor.matmul(
                            pl[:, 512:1024], identbf[:],
                            bt[:, jt0 + 2:jt0 + 4, :], start=True, stop=False,
                        )
                        for jj in range(4):
                            nc.tensor.matmul(
                                pl[:, jj * SHARD:(jj + 1) * SHARD],
                                kh[:, (jt0 + jj) * P:(jt0 + jj + 1) * P],
                                qh[b][:, h, :], start=False,
                                stop=(jj == 1 or jj == 3),
                            )
                        pr = prpool.tile([P, 1024], BF16, tag="pr")
                        nc.scalar.activation(pr[:], pl[:], AF.Exp, bias=zeroc[:, 0:1])
                        for jj in range(4):
                            dst = poA if jj % 2 == 0 else poB
                            jt = jt0 + jj
                            nc.tensor.matmul(
                                dst[:], v33[b][:, jt, h * 33:(h + 1) * 33],
                                pr[:, jj * SHARD:(jj + 1) * SHARD],
                                start=(jt <= 1), stop=(jt >= JT - 2),
                            )
                    ps = smpool.tile([2 * D, SHARD], F32, tag="ps")
                    nc.vector.memset(ps[D:2 * D, :], 0.0)
                    nc.vector.tensor_copy(ps[0:D + 1, :], poA[:])
                    nc.vector.tensor_add(ps[0:D + 1, :], ps[0:D + 1, :], poB[:])
                    nc.vector.reciprocal(ps[D:D + 1, :], ps[D:D + 1, :])
                    rcp = popool.tile([D + 1, SHARD], F32, tag="po", name="rcp")
                    nc.tensor.matmul(
                        rcp[0:D, 0:SHARD], erow[D:2 * D, 0:D], ps[D:2 * D, :],
                        start=True, stop=True,
                    )
                    tmp = smpool.tile([D, SHARD], F32, tag="tmp")
                    nc.vector.tensor_tensor(
                        tmp[:], ps[0:D, :], rcp[0:D, :], ALU.mult
                    )
                    nc.vector.tensor_tensor(
                        outg[b][ro:ro + D, kt_i, :], tmp[:], gh[b][:, h, :], ALU.mult,
                    )

            # ---- phase D: output projection + bo ----
            for b in range(B):
                for m in range(SHARD // P):
                    pf = plpool.tile([P, 1024], F32, tag="pl", name="pf")
                    for kt in range(KT):
                        nc.tensor.matmul(
                            pf[:, 0:E], outg[b][:, kt, m * P:(m + 1) * P],
                            wo[:, kt, :], start=(kt == 0), stop=False,
                        )
                    nc.tensor.matmul(
                        pf[:, 0:E], erowbf[:], bo32[:], start=False, stop=True
                    )
                    fin = fpool.tile([P, E], F32, tag="fin")
                    nc.vector.tensor_copy(fin[:], pf[:, 0:E])
                    nc.sync.dma_start(out=out_d[b, m * P:(m + 1) * P, :], in_=fin[:])

    nc.finalize()
    return nc


def _get_nc():
    if "nc" not in _CACHE:
        _CACHE["nc"] = _build()
    return _CACHE["nc"]


def kernel(x, mask, attn_bias, Wq, Wk, Wv, Wg, bg, Wo, bo, **run_kwargs):
    nc = _get_nc()
    sc = 1.0 / np.sqrt(D)
    bf = ml_dtypes.bfloat16
    x = np.ascontiguousarray(x, np.float32)
    # partition-major relayout: [B*S, E] -> [P, B*JT, E] (row-tile r, partition p)
    x_full = np.ascontiguousarray(
        x.reshape(B * JT, P, E).transpose(1, 0, 2)
    ).reshape(P, B * JT * E)
    maskT = np.ascontiguousarray(
        np.asarray(mask, np.float32).reshape(B, JT, P).transpose(2, 0, 1).reshape(
            P, B * JT
        )
    )
    def _pmaj(w):
        # [E, M] -> [P, ET*M] partition-major
        return np.ascontiguousarray(
            w.reshape(ET, P, w.shape[1]).transpose(1, 0, 2).reshape(P, -1)
        )

    wqt = _pmaj((np.asarray(Wq, np.float32).T * sc).astype(bf))
    wkt = _pmaj(np.asarray(Wk, np.float32).T.astype(bf))
    wvt = _pmaj(np.asarray(Wv, np.float32).T.astype(bf))
    wgt = _pmaj(np.asarray(Wg, np.float32).T.astype(bf))
    wot = _pmaj(np.asarray(Wo, np.float32).T.astype(bf))
    bg = np.ascontiguousarray(bg, np.float32)
    erow_np = np.zeros((D, P), np.float32)
    erow_np[0, :] = 1.0
    erowbf_np = erow_np.astype(bf)
    bo32_np = np.zeros((D, E), np.float32)
    bo32_np[0, :] = np.asarray(bo, np.float32)
    bo32bf_np = bo32_np.astype(bf)

    in_maps = []
    for c in range(NC):
        i0 = c * SHARD
        # [B,H,i-shard,j] -> [B,H,p,(g,i)] so each partition's data is contiguous
        biasT_c = np.ascontiguousarray(
            np.asarray(attn_bias, np.float32)[:, :, i0:i0 + SHARD, :]
            .transpose(0, 1, 3, 2)            # [B,H,j,i]
            .reshape(B, H, JT, P, SHARD)      # j = g*128 + p
            .transpose(0, 1, 3, 2, 4)         # [B,H,p,g,i]
            .astype(bf)
        ).reshape(B, H, P, JT * SHARD)
        xq_c = np.ascontiguousarray(
            x[:, i0:i0 + SHARD, :].reshape(B * 2, P, E).transpose(1, 0, 2)
        ).reshape(P, B * 2 * E)
        in_maps.append({
            "x": x_full, "xq": xq_c, "maskT": maskT, "biasT": biasT_c,
            "wqt": wqt, "wkt": wkt, "wvt": wvt, "wgt": wgt, "wot": wot,
            "bg": bg, "ident": np.eye(P, dtype=np.float32),
            "identbf": np.eye(P, dtype=np.float32).astype(bf),
            "erow": erow_np, "erowbf": erowbf_np, "bo32bf": bo32bf_np,
        })

    res = run_bass_kernel_spmd(nc, in_maps, core_ids=list(range(NC)), **run_kwargs)
    _CACHE["last_results"] = res
    out = np.concatenate([res.results[c]["out"] for c in range(NC)], axis=1)
    return np.ascontiguousarray(out, np.float32)


# revision 37
# speedup vs baseline: 1.0619x; 1.0619x over previous
"""Trainium2 Bass kernel for gated multi-head attention (nn_Attention_19490561589255).

Reference computation (B=2, S=2048, E=256, H=8, D=32):
    xn = LayerNorm(x)                       (no affine)
    q,k,v = split_heads(xn @ W{q,k,v}.T);  q *= 1/sqrt(D)
    gate  = sigmoid(split_heads(xn @ Wg.T + bg))
    logits = q @ k.T + attn_bias;  logits[mask<0.5 keys] = -2^15
    out = gate * softmax(logits) @ v;  out = merge_heads(out) @ Wo.T + bo

Sharding: each of the 8 cores owns a 256-row query block (all batches, all
heads).  k/v/gate weights + full x are replicated (tiny); the 268 MB
attn_bias is sliced by query block and passed pre-transposed so logits are
computed as logitsT[j, i] (keys on partitions).  In that layout:
  - the key mask is applied by zeroing masked rows of the [v | ones]
    stationary operand, so masked keys contribute 0 to both the attn@v
    numerator and the softmax denominator (exactly the -inf semantics),
  - the softmax denominator falls out of the attn@v matmul via the extra
    ones-column (no max-subtraction needed: logits ~ N(0,2), f32-exp safe),
  - attn_bias is injected into PSUM by an identity-stationary matmul so the
    PE accumulation hardware performs the big bias add; adjacent key tiles
    are paired into one [128,512] PSUM bank (one inject + one exp per pair),
  - matmul operands are bf16 (PSUM accumulation stays f32),
  - each core writes its own output rows: NO collectives, host gather is a
    pure concatenation.
"""

import os
import sys

sys.path.insert(0, "/opt/trn_rl_repo")

import ml_dtypes
import numpy as np

import concourse.bass as bass
import concourse.mybir as mybir
import concourse.tile as tile
from concourse import bacc
from concourse.bass_utils import run_bass_kernel_spmd

F32 = mybir.dt.float32
BF16 = mybir.dt.bfloat16
AF = mybir.ActivationFunctionType
ALU = mybir.AluOpType

B, S, E = 2, 2048, 256
H, D = 8, 32
HD = H * D          # 256
P = 128
NC = 8
SHARD = S // NC     # 256 query rows per core
ET = E // P         # 2 e-tiles
KT = HD // P        # 2 hd-tiles
JT = S // P         # 16 key tiles
INF = 2.0 ** 15
EPS = 1e-5

_CACHE = {}


def _build():
    nc = bacc.Bacc(
        "TRN2", target_bir_lowering=False, debug=False, enable_asserts=False,
        num_devices=NC,
    )
    x_d = nc.dram_tensor("x", [P, B * JT * E], F32, kind="ExternalInput").ap()
    xq_d = nc.dram_tensor("xq", [P, B * 2 * E], F32, kind="ExternalInput").ap()
    mk_d = nc.dram_tensor("maskT", [P, B * JT], F32, kind="ExternalInput").ap()
    bias_d = nc.dram_tensor("biasT", [B, H, P, JT * SHARD], BF16, kind="ExternalInput").ap()
    wq_d = nc.dram_tensor("wqt", [P, ET * HD], BF16, kind="ExternalInput").ap()
    wk_d = nc.dram_tensor("wkt", [P, ET * HD], BF16, kind="ExternalInput").ap()
    wv_d = nc.dram_tensor("wvt", [P, ET * HD], BF16, kind="ExternalInput").ap()
    wg_d = nc.dram_tensor("wgt", [P, ET * HD], BF16, kind="ExternalInput").ap()
    wo_d = nc.dram_tensor("wot", [P, KT * E], BF16, kind="ExternalInput").ap()
    bg_d = nc.dram_tensor("bg", [HD], F32, kind="ExternalInput").ap()
    id_d = nc.dram_tensor("ident", [P, P], F32, kind="ExternalInput").ap()
    idb_d = nc.dram_tensor("identbf", [P, P], BF16, kind="ExternalInput").ap()
    er_d = nc.dram_tensor("erow", [D, P], F32, kind="ExternalInput").ap()
    erb_d = nc.dram_tensor("erowbf", [D, P], BF16, kind="ExternalInput").ap()
    bo32_d = nc.dram_tensor("bo32bf", [D, E], BF16, kind="ExternalInput").ap()
    out_d = nc.dram_tensor("out", [B, SHARD, E], F32, kind="ExternalOutput").ap()

    from contextlib import ExitStack

    with ExitStack() as ctx:
        tc = ctx.enter_context(tile.TileContext(nc))
        cpool = ctx.enter_context(tc.tile_pool(name="consts", bufs=1))
        wpool = ctx.enter_context(tc.tile_pool(name="wts", bufs=1))
        xpool = ctx.enter_context(tc.tile_pool(name="xload", bufs=4))
        spool = ctx.enter_context(tc.tile_pool(name="stats", bufs=8))
        xnpool = ctx.enter_context(tc.tile_pool(name="xn", bufs=4))
        xnTpool = ctx.enter_context(tc.tile_pool(name="xnT", bufs=2))
        kTpool = ctx.enter_context(tc.tile_pool(name="kT", bufs=2))
        vpool = ctx.enter_context(tc.tile_pool(name="v33", bufs=2))
        qgpool = ctx.enter_context(tc.tile_pool(name="qg", bufs=2))
        bpool = ctx.enter_context(tc.tile_pool(name="bias", bufs=4))
        prpool = ctx.enter_context(tc.tile_pool(name="probs", bufs=5))
        ogpool = ctx.enter_context(tc.tile_pool(name="outg", bufs=2))
        fpool = ctx.enter_context(tc.tile_pool(name="fin", bufs=2))
        khpool = ctx.enter_context(tc.tile_pool(name="khst", bufs=2))
        smpool = ctx.enter_context(tc.tile_pool(name="small", bufs=2))
        plpool = ctx.enter_context(tc.tile_pool(name="pl", bufs=2, space="PSUM"))
        popool = ctx.enter_context(tc.tile_pool(name="po", bufs=4, space="PSUM"))
        if True:
            # ---- constants / weights ----
            ident = cpool.tile([P, P], F32)
            nc.sync.dma_start(out=ident[:], in_=id_d[:])
            identbf = cpool.tile([P, P], BF16, tag="identbf")
            nc.sync.dma_start(out=identbf[:], in_=idb_d[:])
            erow = cpool.tile([2 * D, P], F32, tag="erow")
            nc.sync.dma_start(out=erow[D:2 * D, :], in_=er_d[:])
            erowbf = cpool.tile([D, P], BF16, tag="erowbf")
            nc.sync.dma_start(out=erowbf[:], in_=erb_d[:])
            bo32 = cpool.tile([D, E], BF16, tag="bo32")
            nc.sync.dma_start(out=bo32[:], in_=bo32_d[:])
            bgp = cpool.tile([D, H], F32)
            nc.sync.dma_start(out=bgp[:], in_=bg_d.rearrange("(h p) -> p h", p=D))
            epsc = cpool.tile([P, 1], F32, tag="epsc")
            nc.vector.memset(epsc[:], EPS)
            zeroc = cpool.tile([P, 1], F32, tag="zeroc")
            nc.vector.memset(zeroc[:], 0.0)

            wq = wpool.tile([P, ET, HD], BF16, tag="wq")
            wk = wpool.tile([P, ET, HD], BF16, tag="wk")
            wv = wpool.tile([P, ET, HD], BF16, tag="wv")
            wg = wpool.tile([P, ET, HD], BF16, tag="wg")
            wo = wpool.tile([P, KT, E], BF16, tag="wo")
            for t, d in ((wq, wq_d), (wk, wk_d), (wv, wv_d), (wg, wg_d), (wo, wo_d)):
                nc.sync.dma_start(out=t[:], in_=d.rearrange("p (et m) -> p et m", m=HD))

            # key-mask multiplier: 1.0 where mask>=0.5 else 0.0
            mk = cpool.tile([P, B * JT], F32, tag="mk")
            nc.sync.dma_start(out=mk[:], in_=mk_d[:])
            maskm = cpool.tile([P, B * JT], F32, tag="maskm")
            nc.vector.tensor_scalar(maskm[:], mk[:], 0.5, None, ALU.is_ge)

            def layernorm_rows(src_groups, dst_tile):
                """LN groups of [128, G, E] rows, transpose into dst_tile."""
                for gi, (src_ap, g) in enumerate(src_groups):
                    xt = xpool.tile([P, 4, E], F32, tag="xt")
                    nc.sync.dma_start(out=xt[:, 0:g, :], in_=src_ap)
                    s1 = spool.tile([P, 4], F32, tag="s1")
                    s2 = spool.tile([P, 4], F32, tag="s2")
                    sq = xpool.tile([P, 4, E], F32, tag="sq")
                    nc.vector.tensor_reduce(
                        s1[:, 0:g], xt[:, 0:g, :], axis=mybir.AxisListType.X,
                        op=ALU.add,
                    )
                    for r in range(g):
                        nc.scalar.activation(
                            sq[:, r, :], xt[:, r, :], AF.Square,
                            accum_out=s2[:, r:r + 1],
                        )
                    mu = spool.tile([P, 4], F32, tag="mu")
                    nc.vector.tensor_scalar_mul(mu[:, 0:g], s1[:, 0:g], 1.0 / E)
                    ex2 = spool.tile([P, 4], F32, tag="ex2")
                    nc.vector.tensor_scalar_mul(ex2[:, 0:g], s2[:, 0:g], 1.0 / E)
                    nvar = spool.tile([P, 4], F32, tag="nvar")
                    # nvar = mu*mu - ex2  (negated variance)
                    nc.vector.tensor_tensor(
                        nvar[:, 0:g], mu[:, 0:g], mu[:, 0:g], ALU.mult
                    )
                    nc.vector.tensor_sub(nvar[:, 0:g], nvar[:, 0:g], ex2[:, 0:g])
                    sd = spool.tile([P, 4], F32, tag="sd")
                    # sd = sqrt(-nvar + eps) = sqrt(var + eps)
                    nc.scalar.activation(
                        sd[:, 0:g], nvar[:, 0:g], AF.Sqrt, bias=epsc[:, 0:1],
                        scale=-1.0,
                    )
                    rs = spool.tile([P, 4], F32, tag="rs")
                    nc.vector.reciprocal(rs[:, 0:g], sd[:, 0:g])
                    xnt = xnpool.tile([P, 4, E], BF16, tag="xnt")
                    # xn = (x - mu) * rs   (free-axis broadcast of [P,g] stats)
                    xc = xpool.tile([P, 4, E], F32, tag="xc")
                    nc.vector.tensor_sub(
                        xc[:, 0:g, :], xt[:, 0:g, :],
                        mu[:, 0:g, None].broadcast_to([P, g, E]),
                    )
                    nc.vector.tensor_mul(
                        xnt[:, 0:g, :], xc[:, 0:g, :],
                        rs[:, 0:g, None].broadcast_to([P, g, E]),
                    )
                    for r in range(g):
                        col = (gi * 4 + r) * P
                        for et in range(ET):
                            pt = plpool.tile([P, 1024], BF16, tag="pl", name="pt")
                            nc.tensor.transpose(
                                pt[:, 0:P], xnt[:, r, et * P:(et + 1) * P],
                                identbf[:],
                            )
                            nc.scalar.copy(
                                dst_tile[:, et, col:col + P], pt[:, 0:P]
                            )

            x_rows = x_d.rearrange("p (r e) -> p r e", e=E)
            xq_rows = xq_d.rearrange("p (r e) -> p r e", e=E)

            # ---- phases A+B per batch: LN+transpose then projections ----
            xnT = {}
            xqnT = {}
            kT = {}
            v33 = {}
            qh = {}
            gh = {}
            for b in range(B):
                xnT[b] = xnTpool.tile([P, ET, S], BF16, tag="xnT", name="xnT")
                layernorm_rows(
                    [(x_rows[:, b * JT + 4 * gi:b * JT + 4 * (gi + 1), :], 4)
                     for gi in range(JT // 4)],
                    xnT[b],
                )
                xqnT[b] = xnTpool.tile([P, ET, SHARD], BF16, tag="xqnT", name="xqnT")
                layernorm_rows(
                    [(xq_rows[:, b * 2:b * 2 + 2, :], 2)],
                    xqnT[b],
                )

                kT[b] = kTpool.tile([P, KT, S], BF16, tag="kT", name="kT")
                for kt in range(KT):
                    for nb in range(S // 512):
                        pp = plpool.tile([P, 1024], F32, tag="pl", name="pp")
                        for et in range(ET):
                            nc.tensor.matmul(
                                pp[:, 0:512], wk[:, et, kt * P:(kt + 1) * P],
                                xnT[b][:, et, nb * 512:(nb + 1) * 512],
                                start=(et == 0), stop=(et == ET - 1),
                            )
                        nc.vector.tensor_copy(
                            kT[b][:, kt, nb * 512:(nb + 1) * 512], pp[:, 0:512]
                        )

                v33[b] = vpool.tile([P, JT, H * 33], BF16, tag="v", name="v33")
                for jt in range(JT):
                    mcol = b * JT + jt
                    pp = plpool.tile([P, 1024], F32, tag="pl", name="pp")
                    for et in range(ET):
                        nc.tensor.matmul(
                            pp[:, 0:HD], xnT[b][:, et, jt * P:(jt + 1) * P],
                            wv[:, et, :], start=(et == 0), stop=(et == ET - 1),
                        )
                    v3 = v33[b][:, jt, :].rearrange("p (h x) -> p h x", x=33)
                    # masked v into cols 0..31 (ACT copy scaled by mask multiplier)
                    nc.scalar.activation(
                        v3[:, :, 0:D],
                        pp[:, 0:HD].rearrange("p (h x) -> p h x", x=D),
                        AF.Copy, scale=maskm[:, mcol:mcol + 1],
                    )
                    # trailing ones-column = mask multiplier (1 live / 0 masked)
                    nc.vector.tensor_copy(
                        v3[:, :, D:D + 1],
                        maskm[:, None, mcol:mcol + 1].broadcast_to([P, H, 1]),
                    )

                qh[b] = qgpool.tile([D, H, SHARD], BF16, tag="q", name="qh")
                gh[b] = qgpool.tile([D, H, SHARD], BF16, tag="g", name="gh")
                for h in range(H):
                    pp = plpool.tile([P, 1024], F32, tag="pl", name="pp")
                    for et in range(ET):
                        nc.tensor.matmul(
                            pp[0:D, 0:SHARD], wq[:, et, h * D:(h + 1) * D],
                            xqnT[b][:, et, :], start=(et == 0), stop=(et == ET - 1),
                        )
                    nc.scalar.copy(qh[b][:, h, :], pp[0:D, 0:SHARD])
                    pp2 = plpool.tile([P, 1024], F32, tag="pl", name="pp2")
                    for et in range(ET):
                        nc.tensor.matmul(
                            pp2[0:D, 0:SHARD], wg[:, et, h * D:(h + 1) * D],
                            xqnT[b][:, et, :], start=(et == 0), stop=(et == ET - 1),
                        )
                    nc.scalar.activation(
                        gh[b][:, h, :], pp2[0:D, 0:SHARD], AF.Sigmoid,
                        bias=bgp[:, h:h + 1],
                    )

            # ---- phase C: attention ----
            outg = {}
            for b in range(B):
                outg[b] = ogpool.tile([P, KT, SHARD], BF16, tag="og", name="outg")
                bias_rows = bias_d[b].rearrange("h p (g i) -> h p g i", i=SHARD)
                for h in range(H):
                    kt_i, ro = h // 4, (h % 4) * D
                    kh = khpool.tile([D, S], BF16, tag="kh")
                    nc.vector.tensor_copy(kh[:], kT[b][ro:ro + D, kt_i, :])
                    bt = bpool.tile([P, JT, SHARD], BF16, tag="bias")
                    nc.gpsimd.dma_start(out=bt[:], in_=bias_rows[h])
                    poA = popool.tile([D + 1, SHARD], F32, tag="po", name="poA")
                    poB = popool.tile([D + 1, SHARD], F32, tag="po", name="poB")
                    for qd in range(JT // 4):
                        jt0 = qd * 4
                        pl = plpool.tile([P, 1024], F32, tag="pl")
                        nc.tensor.matmul(
                            pl[:, 0:512], identbf[:],
                            bt[:, jt0:jt0 + 2, :], start=True, stop=False,
                        )
                        nc.tensor.matmul(
                            pl[:, 512:1024], identbf[:],
                            bt[:, jt0 + 2:jt0 + 4, :], start=True, stop=False,
                        )
                        for jj in range(4):
                            nc.tensor.matmul(
                                pl[:, jj * SHARD:(jj + 1) * SHARD],
                                kh[:, (jt0 + jj) * P:(jt0 + jj + 1) * P],
                                qh[b][:, h, :], start=False,
                                stop=(jj == 1 or jj == 3),
                            )
                        pr = prpool.tile([P, 1024], BF16, tag="pr")
                        nc.scalar.activation(pr[:], pl[:], AF.Exp, bias=zeroc[:, 0:1])
                        for jj in range(4):
                            dst = poA if jj % 2 == 0 else poB
                            jt = jt0 + jj
                            nc.tensor.matmul(
                                dst[:], v33[b][:, jt, h * 33:(h + 1) * 33],
                                pr[:, jj * SHARD:(jj + 1) * SHARD],
                                start=(jt <= 1), stop=(jt >= JT - 2),
                            )
                    ps = smpool.tile([2 * D, SHARD], F32, tag="ps")
                    nc.vector.memset(ps[D:2 * D, :], 0.0)
                    nc.vector.tensor_copy(ps[0:D + 1, :], poA[:])
                    nc.vector.tensor_add(ps[0:D + 1, :], ps[0:D + 1, :], poB[:])
                    nc.vector.reciprocal(ps[D:D + 1, :], ps[D:D + 1, :])
                    rcp = popool.tile([D + 1, SHARD], F32, tag="po", name="rcp")
                    nc.tensor.matmul(
                        rcp[0:D, 0:SHARD], erow[D:2 * D, 0:D], ps[D:2 * D, :],
                        start=True, stop=True,
                    )
                    tmp = smpool.tile([D, SHARD], F32, tag="tmp")
                    nc.vector.tensor_tensor(
                        tmp[:], ps[0:D, :], rcp[0:D, :], ALU.mult
                    )
                    nc.vector.tensor_tensor(
                        outg[b][ro:ro + D, kt_i, :], tmp[:], gh[b][:, h, :], ALU.mult,
                    )

            # ---- phase D: output projection + bo ----
            for b in range(B):
                for m in range(SHARD // P):
                    pf = plpool.tile([P, 1024], F32, tag="pl", name="pf")
                    for kt in range(KT):
                        nc.tensor.matmul(
                            pf[:, 0:E], outg[b][:, kt, m * P:(m + 1) * P],
                            wo[:, kt, :], start=(kt == 0), stop=False,
                        )
                    nc.tensor.matmul(
                        pf[:, 0:E], erowbf[:], bo32[:], start=False, stop=True
                    )
                    fin = fpool.tile([P, E], F32, tag="fin")
                    nc.vector.tensor_copy(fin[:], pf[:, 0:E])
                    nc.sync.dma_start(out=out_d[b, m * P:(m + 1) * P, :], in_=fin[:])

    nc.finalize()
    return nc


def _get_nc():
    if "nc" not in _CACHE:
        _CACHE["nc"] = _build()
    return _CACHE["nc"]


def kernel(x, mask, attn_bias, Wq, Wk, Wv, Wg, bg, Wo, bo, **run_kwargs):
    nc = _get_nc()
    sc = 1.0 / np.sqrt(D)
    bf = ml_dtypes.bfloat16
    x = np.ascontiguousarray(x, np.float32)
    # partition-major relayout: [B*S, E] -> [P, B*JT, E] (row-tile r, partition p)
    x_full = np.ascontiguousarray(
        x.reshape(B * JT, P, E).transpose(1, 0, 2)
    ).reshape(P, B * JT * E)
    maskT = np.ascontiguousarray(
        np.asarray(mask, np.float32).reshape(B, JT, P).transpose(2, 0, 1).reshape(
            P, B * JT
        )
    )
    def _pmaj(w):
        # [E, M] -> [P, ET*M] partition-major
        return np.ascontiguousarray(
            w.reshape(ET, P, w.shape[1]).transpose(1, 0, 2).reshape(P, -1)
        )

    wqt = _pmaj((np.asarray(Wq, np.float32).T * sc).astype(bf))
    wkt = _pmaj(np.asarray(Wk, np.float32).T.astype(bf))
    wvt = _pmaj(np.asarray(Wv, np.float32).T.astype(bf))
    wgt = _pmaj(np.asarray(Wg, np.float32).T.astype(bf))
    wot = _pmaj(np.asarray(Wo, np.float32).T.astype(bf))
    bg = np.ascontiguousarray(bg, np.float32)
    erow_np = np.zeros((D, P), np.float32)
    erow_np[0, :] = 1.0
    erowbf_np = erow_np.astype(bf)
    bo32_np = np.zeros((D, E), np.float32)
    bo32_np[0, :] = np.asarray(bo, np.float32)
    bo32bf_np = bo32_np.astype(bf)

    in_maps = []
    for c in range(NC):
        i0 = c * SHARD
        # [B,H,i-shard,j] -> [B,H,p,(g,i)] so each partition's data is contiguous
        biasT_c = np.ascontiguousarray(
            np.asarray(attn_bias, np.float32)[:, :, i0:i0 + SHARD, :]
            .transpose(0, 1, 3, 2)            # [B,H,j,i]
            .reshape(B, H, JT, P, SHARD)      # j = g*128 + p
            .transpose(0, 1, 3, 2, 4)         # [B,H,p,g,i]
            .astype(bf)
        ).reshape(B, H, P, JT * SHARD)
        xq_c = np.ascontiguousarray(
            x[:, i0:i0 + SHARD, :].reshape(B * 2, P, E).transpose(1, 0, 2)
        ).reshape(P, B * 2 * E)
        in_maps.append({
            "x": x_full, "xq": xq_c, "maskT": maskT, "biasT": biasT_c,
            "wqt": wqt, "wkt": wkt, "wvt": wvt, "wgt": wgt, "wot": wot,
            "bg": bg, "ident": np.eye(P, dtype=np.float32),
            "identbf": np.eye(P, dtype=np.float32).astype(bf),
            "erow": erow_np, "erowbf": erowbf_np, "bo32bf": bo32bf_np,
        })

    res = run_bass_kernel_spmd(nc, in_maps, core_ids=list(range(NC)), **run_kwargs)
    _CACHE["last_results"] = res
    out = np.concatenate([res.results[c]["out"] for c in range(NC)], axis=1)
    return np.ascontiguousarray(out, np.float32)


# revision 39
# speedup vs baseline: 1.1436x; 1.0769x over previous
"""Trainium2 Bass kernel for gated multi-head attention (nn_Attention_19490561589255).

Reference computation (B=2, S=2048, E=256, H=8, D=32):
    xn = LayerNorm(x)                       (no affine)
    q,k,v = split_heads(xn @ W{q,k,v}.T);  q *= 1/sqrt(D)
    gate  = sigmoid(split_heads(xn @ Wg.T + bg))
    logits = q @ k.T + attn_bias;  logits[mask<0.5 keys] = -2^15
    out = gate * softmax(logits) @ v;  out = merge_heads(out) @ Wo.T + bo

Sharding: each of the 8 cores owns a 256-row query block (all batches, all
heads).  k/v/gate weights + full x are replicated (tiny); the 268 MB
attn_bias is sliced by query block and passed pre-transposed so logits are
computed as logitsT[j, i] (keys on partitions).  In that layout:
  - the key mask is applied by zeroing masked rows of the [v | ones]
    stationary operand, so masked keys contribute 0 to both the attn@v
    numerator and the softmax denominator (exactly the -inf semantics),
  - the softmax denominator falls out of the attn@v matmul via the extra
    ones-column (no max-subtraction needed: logits ~ N(0,2), f32-exp safe),
  - attn_bias is injected into PSUM by an identity-stationary matmul so the
    PE accumulation hardware performs the big bias add; adjacent key tiles
    are paired into one [128,512] PSUM bank (one inject + one exp per pair),
  - matmul operands are bf16 (PSUM accumulation stays f32),
  - each core writes its own output rows: NO collectives, host gather is a
    pure concatenation.
"""

import os
import sys

sys.path.insert(0, "/opt/trn_rl_repo")

import ml_dtypes
import numpy as np

import concourse.bass as bass
import concourse.mybir as mybir
import concourse.tile as tile
from concourse import bacc
from concourse.bass_utils import run_bass_kernel_spmd

F32 = mybir.dt.float32
BF16 = mybir.dt.bfloat16
AF = mybir.ActivationFunctionType
ALU = mybir.AluOpType

B, S, E = 2, 2048, 256
H, D = 8, 32
HD = H * D          # 256
P = 128
NC = 8
SHARD = S // NC     # 256 query rows per core
ET = E // P         # 2 e-tiles
KT = HD // P        # 2 hd-tiles
JT = S // P         # 16 key tiles
INF = 2.0 ** 15
EPS = 1e-5

_CACHE = {}


def _build():
    nc = bacc.Bacc(
        "TRN2", target_bir_lowering=False, debug=False, enable_asserts=False,
        num_devices=NC,
    )
    x_d = nc.dram_tensor("x", [P, B * JT * E], F32, kind="ExternalInput").ap()
    xq_d = nc.dram_tensor("xq", [P, B * 2 * E], F32, kind="ExternalInput").ap()
    mk_d = nc.dram_tensor("maskT", [P, B * JT], F32, kind="ExternalInput").ap()
    bias_d = nc.dram_tensor("biasT", [B, H, P, JT * SHARD], BF16, kind="ExternalInput").ap()
    wq_d = nc.dram_tensor("wqt", [P, ET * HD], BF16, kind="ExternalInput").ap()
    wk_d = nc.dram_tensor("wkt", [P, ET * HD], BF16, kind="ExternalInput").ap()
    wv_d = nc.dram_tensor("wvt", [P, ET * HD], BF16, kind="ExternalInput").ap()
    wg_d = nc.dram_tensor("wgt", [P, ET * HD], BF16, kind="ExternalInput").ap()
    wo_d = nc.dram_tensor("wot", [P, KT * E], BF16, kind="ExternalInput").ap()
    bg_d = nc.dram_tensor("bg", [HD], F32, kind="ExternalInput").ap()
    id_d = nc.dram_tensor("ident", [P, P], F32, kind="ExternalInput").ap()
    idb_d = nc.dram_tensor("identbf", [P, P], BF16, kind="ExternalInput").ap()
    er_d = nc.dram_tensor("erow", [D, P], F32, kind="ExternalInput").ap()
    erb_d = nc.dram_tensor("erowbf", [D, P], BF16, kind="ExternalInput").ap()
    bo32_d = nc.dram_tensor("bo32bf", [D, E], BF16, kind="ExternalInput").ap()
    out_d = nc.dram_tensor("out", [B, SHARD, E], F32, kind="ExternalOutput").ap()

    from contextlib import ExitStack

    with ExitStack() as ctx:
        tc = ctx.enter_context(tile.TileContext(nc))
        cpool = ctx.enter_context(tc.tile_pool(name="consts", bufs=1))
        wpool = ctx.enter_context(tc.tile_pool(name="wts", bufs=1))
        xpool = ctx.enter_context(tc.tile_pool(name="xload", bufs=4))
        spool = ctx.enter_context(tc.tile_pool(name="stats", bufs=8))
        xnpool = ctx.enter_context(tc.tile_pool(name="xn", bufs=4))
        xnTpool = ctx.enter_context(tc.tile_pool(name="xnT", bufs=2))
        kTpool = ctx.enter_context(tc.tile_pool(name="kT", bufs=2))
        vpool = ctx.enter_context(tc.tile_pool(name="v33", bufs=2))
        qgpool = ctx.enter_context(tc.tile_pool(name="qg", bufs=2))
        bpool = ctx.enter_context(tc.tile_pool(name="bias", bufs=4))
        prpool = ctx.enter_context(tc.tile_pool(name="probs", bufs=4))
        ogpool = ctx.enter_context(tc.tile_pool(name="outg", bufs=2))
        fpool = ctx.enter_context(tc.tile_pool(name="fin", bufs=2))
        khpool = ctx.enter_context(tc.tile_pool(name="khst", bufs=2))
        smpool = ctx.enter_context(tc.tile_pool(name="small", bufs=2))
        plpool = ctx.enter_context(tc.tile_pool(name="pl", bufs=3, space="PSUM"))
        popool = ctx.enter_context(tc.tile_pool(name="po", bufs=2, space="PSUM"))
        if True:
            # ---- constants / weights ----
            ident = cpool.tile([P, P], F32)
            nc.sync.dma_start(out=ident[:], in_=id_d[:])
            identbf = cpool.tile([P, P], BF16, tag="identbf")
            nc.sync.dma_start(out=identbf[:], in_=idb_d[:])
            erow = cpool.tile([2 * D, P], F32, tag="erow")
            nc.sync.dma_start(out=erow[D:2 * D, :], in_=er_d[:])
            erowbf = cpool.tile([D, P], BF16, tag="erowbf")
            nc.sync.dma_start(out=erowbf[:], in_=erb_d[:])
            bo32 = cpool.tile([D, E], BF16, tag="bo32")
            nc.sync.dma_start(out=bo32[:], in_=bo32_d[:])
            bgp = cpool.tile([D, H], F32)
            nc.sync.dma_start(out=bgp[:], in_=bg_d.rearrange("(h p) -> p h", p=D))
            epsc = cpool.tile([P, 1], F32, tag="epsc")
            nc.vector.memset(epsc[:], EPS)
            zeroc = cpool.tile([P, 1], F32, tag="zeroc")
            nc.vector.memset(zeroc[:], 0.0)

            wq = wpool.tile([P, ET, HD], BF16, tag="wq")
            wk = wpool.tile([P, ET, HD], BF16, tag="wk")
            wv = wpool.tile([P, ET, HD], BF16, tag="wv")
            wg = wpool.tile([P, ET, HD], BF16, tag="wg")
            wo = wpool.tile([P, KT, E], BF16, tag="wo")
            for t, d in ((wq, wq_d), (wk, wk_d), (wv, wv_d), (wg, wg_d), (wo, wo_d)):
                nc.sync.dma_start(out=t[:], in_=d.rearrange("p (et m) -> p et m", m=HD))

            # key-mask multiplier: 1.0 where mask>=0.5 else 0.0
            mk = cpool.tile([P, B * JT], F32, tag="mk")
            nc.sync.dma_start(out=mk[:], in_=mk_d[:])
            maskm = cpool.tile([P, B * JT], F32, tag="maskm")
            nc.vector.tensor_scalar(maskm[:], mk[:], 0.5, None, ALU.is_ge)

            def layernorm_rows(src_groups, dst_tile):
                """LN groups of [128, G, E] rows, transpose into dst_tile."""
                for gi, (src_ap, g) in enumerate(src_groups):
                    xt = xpool.tile([P, 4, E], F32, tag="xt")
                    nc.sync.dma_start(out=xt[:, 0:g, :], in_=src_ap)
                    s1 = spool.tile([P, 4], F32, tag="s1")
                    s2 = spool.tile([P, 4], F32, tag="s2")
                    sq = xpool.tile([P, 4, E], F32, tag="sq")
                    nc.vector.tensor_reduce(
                        s1[:, 0:g], xt[:, 0:g, :], axis=mybir.AxisListType.X,
                        op=ALU.add,
                    )
                    for r in range(g):
                        nc.scalar.activation(
                            sq[:, r, :], xt[:, r, :], AF.Square,
                            accum_out=s2[:, r:r + 1],
                        )
                    mu = spool.tile([P, 4], F32, tag="mu")
                    nc.vector.tensor_scalar_mul(mu[:, 0:g], s1[:, 0:g], 1.0 / E)
                    ex2 = spool.tile([P, 4], F32, tag="ex2")
                    nc.vector.tensor_scalar_mul(ex2[:, 0:g], s2[:, 0:g], 1.0 / E)
                    nvar = spool.tile([P, 4], F32, tag="nvar")
                    # nvar = mu*mu - ex2  (negated variance)
                    nc.vector.tensor_tensor(
                        nvar[:, 0:g], mu[:, 0:g], mu[:, 0:g], ALU.mult
                    )
                    nc.vector.tensor_sub(nvar[:, 0:g], nvar[:, 0:g], ex2[:, 0:g])
                    sd = spool.tile([P, 4], F32, tag="sd")
                    # sd = sqrt(-nvar + eps) = sqrt(var + eps)
                    nc.scalar.activation(
                        sd[:, 0:g], nvar[:, 0:g], AF.Sqrt, bias=epsc[:, 0:1],
                        scale=-1.0,
                    )
                    rs = spool.tile([P, 4], F32, tag="rs")
                    nc.vector.reciprocal(rs[:, 0:g], sd[:, 0:g])
                    xnt = xnpool.tile([P, 4, E], BF16, tag="xnt")
                    # xn = (x - mu) * rs   (free-axis broadcast of [P,g] stats)
                    xc = xpool.tile([P, 4, E], F32, tag="xc")
                    nc.vector.tensor_sub(
                        xc[:, 0:g, :], xt[:, 0:g, :],
                        mu[:, 0:g, None].broadcast_to([P, g, E]),
                    )
                    nc.vector.tensor_mul(
                        xnt[:, 0:g, :], xc[:, 0:g, :],
                        rs[:, 0:g, None].broadcast_to([P, g, E]),
                    )
                    for r in range(g):
                        col = (gi * 4 + r) * P
                        for et in range(ET):
                            pt = plpool.tile([P, 1024], BF16, tag="pl", name="pt")
                            nc.tensor.transpose(
                                pt[:, 0:P], xnt[:, r, et * P:(et + 1) * P],
                                identbf[:],
                            )
                            nc.scalar.copy(
                                dst_tile[:, et, col:col + P], pt[:, 0:P]
                            )

            x_rows = x_d.rearrange("p (r e) -> p r e", e=E)
            xq_rows = xq_d.rearrange("p (r e) -> p r e", e=E)

            # ---- phases A+B per batch: LN+transpose then projections ----
            xnT = {}
            xqnT = {}
            kT = {}
            v33 = {}
            qh = {}
            gh = {}
            for b in range(B):
                xnT[b] = xnTpool.tile([P, ET, S], BF16, tag="xnT", name="xnT")
                layernorm_rows(
                    [(x_rows[:, b * JT + 4 * gi:b * JT + 4 * (gi + 1), :], 4)
                     for gi in range(JT // 4)],
                    xnT[b],
                )
                xqnT[b] = xnTpool.tile([P, ET, SHARD], BF16, tag="xqnT", name="xqnT")
                layernorm_rows(
                    [(xq_rows[:, b * 2:b * 2 + 2, :], 2)],
                    xqnT[b],
                )

                kT[b] = kTpool.tile([P, KT, S], BF16, tag="kT", name="kT")
                for kt in range(KT):
                    for nb in range(S // 512):
                        pp = plpool.tile([P, 1024], F32, tag="pl", name="pp")
                        for et in range(ET):
                            nc.tensor.matmul(
                                pp[:, 0:512], wk[:, et, kt * P:(kt + 1) * P],
                                xnT[b][:, et, nb * 512:(nb + 1) * 512],
                                start=(et == 0), stop=(et == ET - 1),
                            )
                        nc.vector.tensor_copy(
                            kT[b][:, kt, nb * 512:(nb + 1) * 512], pp[:, 0:512]
                        )

                v33[b] = vpool.tile([P, JT, H * 33], BF16, tag="v", name="v33")
                for jt in range(JT):
                    mcol = b * JT + jt
                    pp = plpool.tile([P, 1024], F32, tag="pl", name="pp")
                    for et in range(ET):
                        nc.tensor.matmul(
                            pp[:, 0:HD], xnT[b][:, et, jt * P:(jt + 1) * P],
                            wv[:, et, :], start=(et == 0), stop=(et == ET - 1),
                        )
                    v3 = v33[b][:, jt, :].rearrange("p (h x) -> p h x", x=33)
                    # masked v into cols 0..31 (ACT copy scaled by mask multiplier)
                    nc.scalar.activation(
                        v3[:, :, 0:D],
                        pp[:, 0:HD].rearrange("p (h x) -> p h x", x=D),
                        AF.Copy, scale=maskm[:, mcol:mcol + 1],
                    )
                    # trailing ones-column = mask multiplier (1 live / 0 masked)
                    nc.vector.tensor_copy(
                        v3[:, :, D:D + 1],
                        maskm[:, None, mcol:mcol + 1].broadcast_to([P, H, 1]),
                    )

                qh[b] = qgpool.tile([D, H, SHARD], BF16, tag="q", name="qh")
                gh[b] = qgpool.tile([D, H, SHARD], BF16, tag="g", name="gh")
                for h in range(H):
                    pp = plpool.tile([P, 1024], F32, tag="pl", name="pp")
                    for et in range(ET):
                        nc.tensor.matmul(
                            pp[0:D, 0:SHARD], wq[:, et, h * D:(h + 1) * D],
                            xqnT[b][:, et, :], start=(et == 0), stop=(et == ET - 1),
                        )
                    nc.scalar.copy(qh[b][:, h, :], pp[0:D, 0:SHARD])
                    pp2 = plpool.tile([P, 1024], F32, tag="pl", name="pp2")
                    for et in range(ET):
                        nc.tensor.matmul(
                            pp2[0:D, 0:SHARD], wg[:, et, h * D:(h + 1) * D],
                            xqnT[b][:, et, :], start=(et == 0), stop=(et == ET - 1),
                        )
                    nc.scalar.activation(
                        gh[b][:, h, :], pp2[0:D, 0:SHARD], AF.Sigmoid,
                        bias=bgp[:, h:h + 1],
                    )

            # ---- phase C: attention ----
            outg = {}
            for b in range(B):
                outg[b] = ogpool.tile([P, KT, SHARD], BF16, tag="og", name="outg")
                bias_rows = bias_d[b].rearrange("h p (g i) -> h p g i", i=SHARD)
                for h in range(H):
                    kt_i, ro = h // 4, (h % 4) * D
                    kh = khpool.tile([D, S], BF16, tag="kh")
                    nc.vector.tensor_copy(kh[:], kT[b][ro:ro + D, kt_i, :])
                    bt = bpool.tile([P, JT, SHARD], BF16, tag="bias")
                    nc.gpsimd.dma_start(out=bt[:], in_=bias_rows[h])
                    poA = popool.tile([D + 1, SHARD], F32, tag="po", name="poA")
                    poB = popool.tile([D + 1, SHARD], F32, tag="po", name="poB")
                    for qd in range(JT // 4):
                        jt0 = qd * 4
                        pl = plpool.tile([P, 1024], F32, tag="pl")
                        nc.tensor.matmul(
                            pl[:, 0:512], identbf[:],
                            bt[:, jt0:jt0 + 2, :], start=True, stop=False,
                        )
                        nc.tensor.matmul(
                            pl[:, 512:1024], identbf[:],
                            bt[:, jt0 + 2:jt0 + 4, :], start=True, stop=False,
                        )
                        for jj in range(4):
                            nc.tensor.matmul(
                                pl[:, jj * SHARD:(jj + 1) * SHARD],
                                kh[:, (jt0 + jj) * P:(jt0 + jj + 1) * P],
                                qh[b][:, h, :], start=False,
                                stop=(jj == 1 or jj == 3),
                            )
                        pr = prpool.tile([P, 1024], BF16, tag="pr")
                        nc.scalar.activation(pr[:], pl[:], AF.Exp, bias=zeroc[:, 0:1])
                        for jj in range(4):
                            dst = poA if jj % 2 == 0 else poB
                            jt = jt0 + jj
                            nc.tensor.matmul(
                                dst[:], v33[b][:, jt, h * 33:(h + 1) * 33],
                                pr[:, jj * SHARD:(jj + 1) * SHARD],
                                start=(jt <= 1), stop=(jt >= JT - 2),
                            )
                    ps = smpool.tile([2 * D, SHARD], F32, tag="ps")
                    nc.vector.memset(ps[D:2 * D, :], 0.0)
                    nc.vector.tensor_copy(ps[0:D + 1, :], poA[:])
                    nc.vector.tensor_add(ps[0:D + 1, :], ps[0:D + 1, :], poB[:])
                    nc.vector.reciprocal(ps[D:D + 1, :], ps[D:D + 1, :])
                    rcp = popool.tile([D + 1, SHARD], F32, tag="po", name="rcp")
                    nc.tensor.matmul(
                        rcp[0:D, 0:SHARD], erow[D:2 * D, 0:D], ps[D:2 * D, :],
                        start=True, stop=True,
                    )
                    tmp = smpool.tile([D, SHARD], F32, tag="tmp")
                    nc.vector.tensor_tensor(
                        tmp[:], ps[0:D, :], rcp[0:D, :], ALU.mult
                    )
                    nc.vector.tensor_tensor(
                        outg[b][ro:ro + D, kt_i, :], tmp[:], gh[b][:, h, :], ALU.mult,
                    )

            # ---- phase D: output projection + bo ----
            for b in range(B):
                for m in range(SHARD // P):
                    pf = plpool.tile([P, 1024], F32, tag="pl", name="pf")
                    for kt in range(KT):
                        nc.tensor.matmul(
                            pf[:, 0:E], outg[b][:, kt, m * P:(m + 1) * P],
                            wo[:, kt, :], start=(kt == 0), stop=False,
                        )
                    nc.tensor.matmul(
                        pf[:, 0:E], erowbf[:], bo32[:], start=False, stop=True
                    )
                    fin = fpool.tile([P, E], F32, tag="fin")
                    nc.vector.tensor_copy(fin[:], pf[:, 0:E])
                    nc.sync.dma_start(out=out_d[b, m * P:(m + 1) * P, :], in_=fin[:])

    nc.finalize()
    return nc


def _get_nc():
    if "nc" not in _CACHE:
        _CACHE["nc"] = _build()
    return _CACHE["nc"]


def kernel(x, mask, attn_bias, Wq, Wk, Wv, Wg, bg, Wo, bo, **run_kwargs):
    nc = _get_nc()
    sc = 1.0 / np.sqrt(D)
    bf = ml_dtypes.bfloat16
    x = np.ascontiguousarray(x, np.float32)
    # partition-major relayout: [B*S, E] -> [P, B*JT, E] (row-tile r, partition p)
    x_full = np.ascontiguousarray(
        x.reshape(B * JT, P, E).transpose(1, 0, 2)
    ).reshape(P, B * JT * E)
    maskT = np.ascontiguousarray(
        np.asarray(mask, np.float32).reshape(B, JT, P).transpose(2, 0, 1).reshape(
            P, B * JT
        )
    )
    def _pmaj(w):
        # [E, M] -> [P, ET*M] partition-major
        return np.ascontiguousarray(
            w.reshape(ET, P, w.shape[1]).transpose(1, 0, 2).reshape(P, -1)
        )

    wqt = _pmaj((np.asarray(Wq, np.float32).T * sc).astype(bf))
    wkt = _pmaj(np.asarray(Wk, np.float32).T.astype(bf))
    wvt = _pmaj(np.asarray(Wv, np.float32).T.astype(bf))
    wgt = _pmaj(np.asarray(Wg, np.float32).T.astype(bf))
    wot = _pmaj(np.asarray(Wo, np.float32).T.astype(bf))
    bg = np.ascontiguousarray(bg, np.float32)
    erow_np = np.zeros((D, P), np.float32)
    erow_np[0, :] = 1.0
    erowbf_np = erow_np.astype(bf)
    bo32_np = np.zeros((D, E), np.float32)
    bo32_np[0, :] = np.asarray(bo, np.float32)
    bo32bf_np = bo32_np.astype(bf)

    in_maps = []
    for c in range(NC):
        i0 = c * SHARD
        # [B,H,i-shard,j] -> [B,H,p,(g,i)] so each partition's data is contiguous
        biasT_c = np.ascontiguousarray(
            np.asarray(attn_bias, np.float32)[:, :, i0:i0 + SHARD, :]
            .transpose(0, 1, 3, 2)            # [B,H,j,i]
            .reshape(B, H, JT, P, SHARD)      # j = g*128 + p
            .transpose(0, 1, 3, 2, 4)         # [B,H,p,g,i]
            .astype(bf)
        ).reshape(B, H, P, JT * SHARD)
        xq_c = np.ascontiguousarray(
            x[:, i0:i0 + SHARD, :].reshape(B * 2, P, E).transpose(1, 0, 2)
        ).reshape(P, B * 2 * E)
        in_maps.append({
            "x": x_full, "xq": xq_c, "maskT": maskT, "biasT": biasT_c,
            "wqt": wqt, "wkt": wkt, "wvt": wvt, "wgt": wgt, "wot": wot,
            "bg": bg, "ident": np.eye(P, dtype=np.float32),
            "identbf": np.eye(P, dtype=np.float32).astype(bf),
            "erow": erow_np, "erowbf": erowbf_np, "bo32bf": bo32bf_np,
        })

    res = run_bass_kernel_spmd(nc, in_maps, core_ids=list(range(NC)), **run_kwargs)
    _CACHE["last_results"] = res
    out = np.concatenate([res.results[c]["out"] for c in range(NC)], axis=1)
    return np.ascontiguousarray(out, np.float32)
